# revision 50
# baseline (speedup 1.0000x reference)
"""Causal multi-head attention block (B=2, S=2048, D=768, H=12) on 8 trn2 cores.

Sharding: core c -> batch b = c//4 (data parallel), head group g = c%4
(tensor parallel, 3 heads per group). Each core computes its group's QKV
projection, causal attention, and a partial O-projection over its 192
z-columns. Host sums the 4 partials per batch and adds the biases that
commute through the math (v-bias and b_o).

On-core layout (everything "transposed", d on partitions, seq on free):
  xT   [768, 2048]   q/kT  [64*, 2048]      scores^T [keys, q]
so the softmax denominator comes free from a ones-column appended to V in
the PV matmul, and no on-chip transposes of activations are needed except
V (built via PE transpose from V^T).

The QKV projection uses a host-repacked weight matrix so every 128-wide
M-group is fully used:
  m0=[q_h0 q_h1] m1=[q_h2 v_h0] m2=[k_h0 k_h1] m3=[k_h2 v_h1] m4=[v_h2]
(q rows pre-scaled by 1/8; v bias folded into the host-side epilogue).

Matmul operands are float32r (full-rate fp32 on the PE). Scheduling
interleaves projection/transpose/O-proj work into the attention loop so
the scalar engine (exp) is never starved by a long PE FIFO stretch.
"""

import os
from collections import deque
from contextlib import ExitStack

import numpy as np

import concourse.bass as bass
import concourse.tile as tile
from concourse import bacc, mybir
from concourse.bass_utils import run_bass_kernel_spmd
from concourse.masks import make_identity

F32 = mybir.dt.float32
F32R = mybir.dt.float32r
AF = mybir.ActivationFunctionType

B, S, D = 2, 2048, 768
NH, DH = 12, 64
HPC = 3            # heads per core
GD = HPC * DH      # 192 z-cols per core
KT, QT = 128, 512  # key tile (partitions), q tile (psum free)
NKT, NQT = S // KT, S // QT   # 16, 4
NTOK = S // 128    # 16 token tiles
NKD = D // 128     # 6 contraction tiles for the projections
WPK = 2 * GD + GD  # 576 packed projection rows


def build_bass():
    nc = bacc.Bacc(None)
    xT = nc.dram_tensor("xT", [D, S], F32, kind="ExternalInput")
    wpk = nc.dram_tensor("wpk", [D, WPK], F32, kind="ExternalInput")
    woT = nc.dram_tensor("woT", [GD, D], F32, kind="ExternalInput")
    bqk = nc.dram_tensor("bqk", [128, 4], F32, kind="ExternalInput")
    vones = nc.dram_tensor("vones", [128, 64], F32, kind="ExternalInput")
    out_p = nc.dram_tensor("out_p", [S, D], F32, kind="ExternalOutput")

    with tile.TileContext(nc) as tc, ExitStack() as ctx:
        const = ctx.enter_context(tc.tile_pool(name="const", bufs=1))
        ps = ctx.enter_context(tc.tile_pool(name="ps", bufs=6, space="PSUM"))
        psz = ctx.enter_context(tc.tile_pool(name="psz", bufs=2, space="PSUM"))
        expp = ctx.enter_context(tc.tile_pool(name="expp", bufs=8))
        small = ctx.enter_context(tc.tile_pool(name="small", bufs=4))

        xT_sb = const.tile([128, NKD, S], F32R)
        wpk_sb = const.tile([128, NKD, WPK], F32R)
        wo_a = const.tile([128, D], F32R)
        wo_b = const.tile([64, D], F32R)
        bqk_sb = const.tile([128, 4], F32)
        qT_sb = const.tile([128, 2, S], F32R)
        kT_sb = const.tile([128, 2, S], F32R)
        vvT = const.tile([128, 2, S], F32)
        v_aug = const.tile([128, HPC, NKT, DH + 1], F32R)
        zT01 = const.tile([128, S], F32R)
        zT2 = const.tile([64, S], F32R)
        ident = const.tile([128, 128], F32)
        ones64 = const.tile([1, 64], F32R)

        ones_stage = const.tile([128, 64], F32)
        make_identity(nc, ident[:])

        # ---- loads: k-interleaved so the first projection k-pairs unblock
        # early; everything not needed for (h0, qt0) comes after.
        xT_t = xT.rearrange("(t p) s -> t p s", p=128)
        wpk_t = wpk.rearrange("(t p) m -> t p m", p=128)
        for t in range(NKD):
            nc.sync.dma_start(
                out=wpk_sb[:, t, 0:384], in_=wpk_t[t][:, 0:384].bitcast(F32R)
            )
            nc.sync.dma_start(
                out=xT_sb[:, t, 0:QT], in_=xT_t[t][:, 0:QT].bitcast(F32R)
            )
        nc.sync.dma_start(out=bqk_sb[:], in_=bqk[:, :])
        for t in range(NKD):
            nc.sync.dma_start(
                out=wpk_sb[:, t, 384:WPK], in_=wpk_t[t][:, 384:WPK].bitcast(F32R)
            )
        nc.sync.dma_start(out=ones_stage[:], in_=vones[:, :])
        nc.sync.dma_start(out=ones64[:], in_=vones[0:1, 0:64].bitcast(F32R))
        nc.vector.tensor_copy(
            out=v_aug[:, :, :, DH],
            in_=ones_stage[:, 0 : HPC * NKT]
            .rearrange("p (h t) -> p h t", h=HPC)
            .bitcast(F32R),
        )
        for t in range(NKD):
            nc.sync.dma_start(
                out=xT_sb[:, t, QT : 2 * QT], in_=xT_t[t][:, QT : 2 * QT].bitcast(F32R)
            )
        nc.sync.dma_start(out=wo_a[:], in_=woT[0:128, :].bitcast(F32R))
        nc.sync.dma_start(out=wo_b[:], in_=woT[128:GD, :].bitcast(F32R))
        for t in range(NKD):
            nc.sync.dma_start(
                out=xT_sb[:, t, 2 * QT : S], in_=xT_t[t][:, 2 * QT : S].bitcast(F32R)
            )

        # packed projection m-groups: (col0, rows, evict spec)
        # evict spec: list of (psum row range, dst ap fn, bias col or None)
        def ev_q(col):
            return lambda n, r0, r1: qT_sb[r0:r1, col, n * QT : (n + 1) * QT]

        def ev_k(col):
            return lambda n, r0, r1: kT_sb[r0:r1, col, n * QT : (n + 1) * QT]

        def ev_v(col):
            return lambda n, r0, r1: vvT[r0:r1, col, n * QT : (n + 1) * QT]

        mgroups = [
            (0, 128, [((0, 128), ev_q(0), 0)]),
            (128, 128, [((0, 64), ev_q(1), 1), ((64, 128), ev_v(0), None)]),
            (256, 128, [((0, 128), ev_k(0), 2)]),
            (384, 128, [((0, 64), ev_k(1), 3), ((64, 128), ev_v(1), None)]),
            (512, 64, [((0, 64), ev_v(0), None)]),
        ]
        # v pieces: v_h0 -> vvT[64:128, 0], v_h1 -> vvT[64:128, 1],
        # v_h2 -> vvT[0:64, 0] (from the m4 group, psum rows 0:64)

        proj_psums = {}

        def proj_unit(mi, n, kpair):
            """Two K-step matmuls of group (mi, n); evictions after the last."""
            c0, msz, evicts = mgroups[mi]
            key = (mi, n)
            if key not in proj_psums:
                proj_psums[key] = ps.tile([128, QT], F32, tag="ps", name="projp")
            p = proj_psums[key]
            for k in (2 * kpair, 2 * kpair + 1):
                nc.tensor.matmul(
                    p[:msz, :],
                    lhsT=wpk_sb[:, k, c0 : c0 + msz],
                    rhs=xT_sb[:, k, n * QT : (n + 1) * QT],
                    start=(k == 0),
                    stop=(k == NKD - 1),
                )
            if kpair == 2:
                del proj_psums[key]
                for (r0, r1), dst, bcol in evicts:
                    if mi == 4:
                        dst_ap = dst(n, 0, 64)  # v_h2 rows live at psum 0:64
                    else:
                        dst_ap = dst(n, r0, r1)
                    if bcol is None:
                        nc.vector.tensor_copy(out=dst_ap, in_=p[r0:r1, :])
                    else:
                        nc.vector.tensor_scalar_add(
                            out=dst_ap,
                            in0=p[r0:r1, :],
                            scalar1=bqk_sb[r0:r1, bcol : bcol + 1],
                        )

        def transpose_unit(t, piece):
            """piece 0/1/2 = head 0/1/2; v_h0/v_h1 at vvT[64:128,0/1], v_h2 at vvT[0:64,0]."""
            if piece == 2:
                src = vvT[0:64, 0, t * 128 : (t + 1) * 128]
                idn = ident[0:64, 0:64]
            else:
                src = vvT[64:128, piece, t * 128 : (t + 1) * 128]
                idn = ident[64:128, 64:128]
            pt = ps.tile([128, QT], F32, tag="ps")
            nc.tensor.transpose(pt[:, 0:64], src, idn)
            nc.vector.tensor_copy(v_aug[:, piece, t, 0:64], pt[:, 0:64])

        out_pair = out_p.rearrange("(tp a p) d -> tp p a d", a=2, p=128)
        o_pairs = {}

        def o_proj_unit(t, n2):
            key = t // 2
            if key not in o_pairs:
                o_pairs[key] = expp.tile([128, 2, D], F32, tag="osb", name="osb", bufs=2)
            ob = o_pairs[key]
            po = ps.tile([128, QT], F32, tag="ps")
            nc.tensor.matmul(
                po[:, 0:384],
                lhsT=zT01[:, t * 128 : (t + 1) * 128],
                rhs=wo_a[:, n2 * 384 : (n2 + 1) * 384],
                start=True,
                stop=False,
            )
            nc.tensor.matmul(
                po[:, 0:384],
                lhsT=zT2[:, t * 128 : (t + 1) * 128],
                rhs=wo_b[:, n2 * 384 : (n2 + 1) * 384],
                start=False,
                stop=True,
            )
            if t >= 12 and (t + n2) % 2 == 0:
                nc.scalar.activation(
                    out=ob[:, t % 2, n2 * 384 : (n2 + 1) * 384],
                    in_=po[:, 0:384],
                    func=AF.Copy,
                )
            else:
                nc.vector.tensor_copy(
                    out=ob[:, t % 2, n2 * 384 : (n2 + 1) * 384], in_=po[:, 0:384]
                )
            if t % 2 == 1 and n2 == 1:
                del o_pairs[key]
                nc.sync.dma_start(out=out_pair[key], in_=ob[:, :, :])

        # background work queue of (key, fn), drained between attention
        # iterations. Queue order is topological (a group's transposes come
        # after its evictions), so force-draining "through the last needed
        # unit" preserves all producer->consumer program ordering.
        work = deque()

        def q_proj(n, mis=range(5)):
            for mi in mis:
                for kpair in range(3):
                    work.append(
                        (("proj", n, mi), lambda mi=mi, n=n, kp=kpair: proj_unit(mi, n, kp))
                    )

        def q_tr(ts, pieces=range(HPC)):
            for t in ts:
                for piece in pieces:
                    work.append(
                        (("tr", t, piece), lambda t=t, p=piece: transpose_unit(t, p))
                    )

        def drain(k=1):
            for _ in range(k):
                if work:
                    work.popleft()[1]()

        def drain_all():
            while work:
                work.popleft()[1]()

        PROJ_GROUPS_FOR_HEAD = {0: (0, 1, 2), 1: (0, 2, 3), 2: (1, 3, 4)}

        def force_drain_for(h, qt):
            """Emit queued units up to the last one attention(h, qt) depends on."""
            needed = set()
            for n in range(qt + 1):
                for mi in PROJ_GROUPS_FOR_HEAD[h]:
                    needed.add(("proj", n, mi))
            for t in range(4 * qt + 4):
                needed.add(("tr", t, h))
            last = -1
            for i, (key, _) in enumerate(work):
                if key in needed:
                    last = i
            for _ in range(last + 1):
                work.popleft()[1]()

        def qh(h):
            m, off = divmod(h * 64, 128)
            return qT_sb[off : off + 64, m, :]

        def kh(h):
            m, off = divmod(h * 64, 128)
            return kT_sb[off : off + 64, m, :]

        zdst = [zT01[0:64, :], zT01[64:128, :], zT2[0:64, :]]

        # PV matmuls are pipelined ~4 iterations behind their exp across
        # block boundaries, so the in-order PE FIFO never waits on the
        # exp/mask chain, not even at the end of a block.
        pvq = deque()  # (block_serial, pv_closure)
        blk_serial = [0]

        def pv_drain(depth):
            while len(pvq) > depth:
                pvq.popleft()[1]()

        def pv_flush(upto_serial):
            while pvq and pvq[0][0] <= upto_serial:
                pvq.popleft()[1]()

        def attention(h, qt, per_kt):
            """scores^T -> exp -> causal mask -> PV into zp; diagonal blocks
            narrowed to q columns >= 128*r."""
            zp = psz.tile([DH + 1, QT], F32)
            nkt = 4 * qt + 4
            blk = blk_serial[0]
            blk_serial[0] += 1

            def pv(kt, es, lo):
                nc.tensor.matmul(
                    zp[:, lo:QT],
                    lhsT=v_aug[:, h, kt, :],
                    rhs=es[:, lo:QT],
                    start=(kt == 0),
                    stop=(kt == nkt - 1),
                )

            for kt in range(nkt):
                rr = kt - 4 * qt
                lo = 128 * rr if rr > 0 else 0
                sp = ps.tile([128, QT], F32, tag="ps")
                nc.tensor.matmul(
                    sp[:, lo:QT],
                    lhsT=kh(h)[:, kt * 128 : (kt + 1) * 128],
                    rhs=qh(h)[:, qt * QT + lo : (qt + 1) * QT],
                    start=True,
                    stop=True,
                )
                es = expp.tile([128, QT], F32R, tag="expp")
                nc.scalar.activation(out=es[:, lo:QT], in_=sp[:, lo:QT], func=AF.Exp)
                if rr >= 0:  # diagonal block: zero where key > query
                    nc.gpsimd.affine_select(
                        out=es[:, lo:QT],
                        in_=es[:, lo:QT],
                        compare_op=mybir.AluOpType.is_ge,
                        fill=0.0,
                        base=0,
                        channel_multiplier=-1,
                        pattern=[[1, QT - lo]],
                    )
                pvq.append((blk, lambda kt=kt, es=es, lo=lo: pv(kt, es, lo)))
                if not (qt == NQT - 1 and h == HPC - 1 and len(work) <= 3):
                    drain(per_kt)
                pv_drain(4)
            return zp, blk

        def normalize(zp, h, qt):
            rec = small.tile([1, QT], F32R, tag="rec")
            with nc.allow_low_precision(reason="f32r is fp32-precision"):
                nc.vector.reciprocal(rec[:], zp[DH : DH + 1, :])
            bc = ps.tile([128, QT], F32, tag="ps")
            nc.tensor.matmul(
                bc[0:64, :], lhsT=ones64[:], rhs=rec[:], start=True, stop=True
            )
            bc_sb = small.tile([64, QT], F32, tag="bcsb")
            nc.scalar.activation(out=bc_sb[:], in_=bc[0:64, :], func=AF.Copy)
            nc.vector.tensor_mul(
                zdst[h][:, qt * QT : (qt + 1) * QT], zp[0:DH, :], bc_sb[:]
            )

        # ---- schedule ----
        # prologue: only what attention(h0, qt0) needs; the rest queues up.
        for mi in (0, 2, 1):
            for kpair in range(3):
                proj_unit(mi, 0, kpair)
        for t in range(4):
            transpose_unit(t, 0)
        q_proj(0, mis=(3,))
        q_tr(range(4), pieces=(1,))
        q_proj(0, mis=(4,))
        q_tr(range(4), pieces=(2,))
        for n in range(1, NQT):
            q_proj(n, mis=(0, 2, 1))
            q_tr(range(4 * n, 4 * n + 4), pieces=(0,))
            q_proj(n, mis=(3,))
            q_tr(range(4 * n, 4 * n + 4), pieces=(1,))
            q_proj(n, mis=(4,))
            q_tr(range(4 * n, 4 * n + 4), pieces=(2,))

        pending = None
        for qt in range(NQT):
            per_kt = 3 if qt == 0 else 1
            for h in range(HPC):
                force_drain_for(h, qt)
                zp, blk = attention(h, qt, per_kt)
                if pending is not None:
                    pv_flush(pending[3])  # pending block's PV accumulation done
                    normalize(*pending[:3])
                    ph, pqt = pending[1], pending[2]
                    if ph == HPC - 1:  # whole q-tile normalized -> O-proj ready
                        for t in range(4 * pqt, 4 * pqt + 4):
                            for n2 in range(2):
                                work.append(
                                    (("o", pqt), lambda t=t, n2=n2: o_proj_unit(t, n2))
                                )
                pending = (zp, h, qt, blk)
        pv_flush(pending[3])
        normalize(*pending[:3])
        drain_all()
        for t in range(4 * (NQT - 1), 4 * NQT):
            for n2 in range(2):
                o_proj_unit(t, n2)
    nc.finalize()
    return nc


_NC_CACHE = {}


def make_in_maps(x, W_qkv, b_qkv, W_o):
    in_maps = []
    for c in range(8):
        b, g = divmod(c, 4)
        hs = [HPC * g + i for i in range(HPC)]
        qr = [np.arange(64 * h, 64 * h + 64) for h in hs]
        w_q = [W_qkv[i] * 0.125 for i in qr]
        w_k = [W_qkv[768 + i] for i in qr]
        w_v = [W_qkv[1536 + i] for i in qr]
        b_q = [b_qkv[i] * 0.125 for i in qr]
        b_k = [b_qkv[768 + i] for i in qr]
        # packed rows: m0=[q0 q1] m1=[q2 v0] m2=[k0 k1] m3=[k2 v1] m4=[v2]
        wpk = np.concatenate(
            [w_q[0], w_q[1], w_q[2], w_v[0], w_k[0], w_k[1], w_k[2], w_v[1], w_v[2]],
            axis=0,
        )
        bqk_col = np.zeros((128, 4), np.float32)
        bqk_col[:, 0] = np.concatenate([b_q[0], b_q[1]])
        bqk_col[0:64, 1] = b_q[2]
        bqk_col[:, 2] = np.concatenate([b_k[0], b_k[1]])
        bqk_col[0:64, 3] = b_k[2]
        in_maps.append(
            {
                "xT": np.ascontiguousarray(x[b].T),
                "wpk": np.ascontiguousarray(wpk.T),
                "woT": np.ascontiguousarray(W_o[:, GD * g : GD * (g + 1)].T),
                "bqk": bqk_col,
                "vones": np.ones((128, 64), np.float32),
            }
        )
    return in_maps


def make_in_maps_for_test(inputs):
    return make_in_maps(
        np.asarray(inputs["x"], np.float32),
        np.asarray(inputs["W_qkv"], np.float32),
        np.asarray(inputs["b_qkv"], np.float32),
        np.asarray(inputs["W_o"], np.float32),
    )


def kernel(x, W_qkv, b_qkv, W_o, b_o):
    x = np.asarray(x, np.float32)
    W_qkv = np.asarray(W_qkv, np.float32)
    b_qkv = np.asarray(b_qkv, np.float32)
    W_o = np.asarray(W_o, np.float32)
    b_o = np.asarray(b_o, np.float32)

    if "nc" not in _NC_CACHE:
        _NC_CACHE["nc"] = build_bass()
    nc = _NC_CACHE["nc"]

    in_maps = make_in_maps(x, W_qkv, b_qkv, W_o)

    res = run_bass_kernel_spmd(
        nc,
        in_maps,
        list(range(8)),
        trace=bool(int(os.environ.get("KERNEL_TRACE", "0"))),
    )
    _NC_CACHE["last_results"] = res

    out = np.zeros((B, S, D), np.float32)
    for c in range(8):
        out[c // 4] += res.results[c]["out_p"]
    out += b_qkv[1536:] @ W_o.T + b_o
    return out


# revision 54
# speedup vs baseline: 1.0236x; 1.0236x over previous
"""Causal multi-head attention block (B=2, S=2048, D=768, H=12) on 8 trn2 cores.

Sharding: core c -> batch b = c//4 (data parallel), head group g = c%4
(tensor parallel, 3 heads per group). Each core computes its group's QKV
projection, causal attention, and a partial O-projection over its 192
z-columns. Host sums the 4 partials per batch and adds the biases that
commute through the math (v-bias and b_o).

On-core layout (everything "transposed", d on partitions, seq on free):
  xT   [768, 2048]   q/kT  [64*, 2048]      scores^T [keys, q]
so the softmax denominator comes free from a ones-column appended to V in
the PV matmul, and no on-chip transposes of activations are needed except
V (built via PE transpose from V^T).

The QKV projection uses a host-repacked weight matrix so every 128-wide
M-group is fully used:
  m0=[q_h0 q_h1] m1=[q_h2 v_h0] m2=[k_h0 k_h1] m3=[k_h2 v_h1] m4=[v_h2]
(q rows pre-scaled by 1/8; v bias folded into the host-side epilogue).

Matmul operands are float32r (full-rate fp32 on the PE). Scheduling
interleaves projection/transpose/O-proj work into the attention loop so
the scalar engine (exp) is never starved by a long PE FIFO stretch.
"""

import os
from collections import deque
from contextlib import ExitStack

import numpy as np

import concourse.tile as tile
from concourse import bacc, mybir
from concourse.bass_utils import run_bass_kernel_spmd
from concourse.masks import make_identity

F32 = mybir.dt.float32
F32R = mybir.dt.float32r
AF = mybir.ActivationFunctionType

B, S, D = 2, 2048, 768
NH, DH = 12, 64
HPC = 3            # heads per core
GD = HPC * DH      # 192 z-cols per core
KT, QT = 128, 512  # key tile (partitions), q tile (psum free)
NKT, NQT = S // KT, S // QT   # 16, 4
NTOK = S // 128    # 16 token tiles
NKD = D // 128     # 6 contraction tiles for the projections
WPK = 2 * GD + GD  # 576 packed projection rows


def build_bass():
    nc = bacc.Bacc(None)
    xT = nc.dram_tensor("xT", [D, S], F32, kind="ExternalInput")
    wpk = nc.dram_tensor("wpk", [D, WPK], F32, kind="ExternalInput")
    woT = nc.dram_tensor("woT", [GD, D], F32, kind="ExternalInput")
    bqk = nc.dram_tensor("bqk", [128, 4], F32, kind="ExternalInput")
    vones = nc.dram_tensor("vones", [128, 64], F32, kind="ExternalInput")
    out_p = nc.dram_tensor("out_p", [S, D], F32, kind="ExternalOutput")

    with tile.TileContext(nc) as tc, ExitStack() as ctx:
        const = ctx.enter_context(tc.tile_pool(name="const", bufs=1))
        ps = ctx.enter_context(tc.tile_pool(name="ps", bufs=6, space="PSUM"))
        psz = ctx.enter_context(tc.tile_pool(name="psz", bufs=2, space="PSUM"))
        expp = ctx.enter_context(tc.tile_pool(name="expp", bufs=8))
        small = ctx.enter_context(tc.tile_pool(name="small", bufs=4))

        xT_sb = const.tile([128, NKD, S], F32R)
        wpk_sb = const.tile([128, NKD, WPK], F32R)
        wo_a = const.tile([128, D], F32R)
        wo_b = const.tile([64, D], F32R)
        bqk_sb = const.tile([128, 4], F32)
        qT_sb = const.tile([128, 2, S], F32R)
        kT_sb = const.tile([128, 2, S], F32R)
        vvT = const.tile([128, 2, S], F32)
        v_aug = const.tile([128, HPC, NKT, DH + 1], F32R)
        zT01 = const.tile([128, S], F32R)
        zT2 = const.tile([64, S], F32R)
        ident = const.tile([128, 128], F32)
        ones64 = const.tile([1, 64], F32R)

        ones_stage = const.tile([128, 64], F32)
        make_identity(nc, ident[:])

        # ---- loads: k-interleaved so the first projection k-pairs unblock
        # early; everything not needed for (h0, qt0) comes after.
        xT_t = xT.rearrange("(t p) s -> t p s", p=128)
        wpk_t = wpk.rearrange("(t p) m -> t p m", p=128)
        for t in range(NKD):
            nc.sync.dma_start(
                out=wpk_sb[:, t, 0:384], in_=wpk_t[t][:, 0:384].bitcast(F32R)
            )
            nc.sync.dma_start(
                out=xT_sb[:, t, 0:QT], in_=xT_t[t][:, 0:QT].bitcast(F32R)
            )
        nc.sync.dma_start(out=bqk_sb[:], in_=bqk[:, :])
        for t in range(NKD):
            nc.sync.dma_start(
                out=wpk_sb[:, t, 384:WPK], in_=wpk_t[t][:, 384:WPK].bitcast(F32R)
            )
        nc.sync.dma_start(out=ones_stage[:], in_=vones[:, :])
        nc.sync.dma_start(out=ones64[:], in_=vones[0:1, 0:64].bitcast(F32R))
        nc.vector.tensor_copy(
            out=v_aug[:, :, :, DH],
            in_=ones_stage[:, 0 : HPC * NKT]
            .rearrange("p (h t) -> p h t", h=HPC)
            .bitcast(F32R),
        )
        for t in range(NKD):
            nc.sync.dma_start(
                out=xT_sb[:, t, QT : 2 * QT], in_=xT_t[t][:, QT : 2 * QT].bitcast(F32R)
            )
        nc.sync.dma_start(out=wo_a[:], in_=woT[0:128, :].bitcast(F32R))
        nc.sync.dma_start(out=wo_b[:], in_=woT[128:GD, :].bitcast(F32R))
        for t in range(NKD):
            nc.sync.dma_start(
                out=xT_sb[:, t, 2 * QT : S], in_=xT_t[t][:, 2 * QT : S].bitcast(F32R)
            )

        # packed projection m-groups: (col0, rows, evict spec)
        # evict spec: list of (psum row range, dst ap fn, bias col or None)
        def ev_q(col):
            return lambda n, r0, r1: qT_sb[r0:r1, col, n * QT : (n + 1) * QT]

        def ev_k(col):
            return lambda n, r0, r1: kT_sb[r0:r1, col, n * QT : (n + 1) * QT]

        def ev_v(col):
            return lambda n, r0, r1: vvT[r0:r1, col, n * QT : (n + 1) * QT]

        mgroups = [
            (0, 128, [((0, 128), ev_q(0), 0)]),
            (128, 128, [((0, 64), ev_q(1), 1), ((64, 128), ev_v(0), None)]),
            (256, 128, [((0, 128), ev_k(0), 2)]),
            (384, 128, [((0, 64), ev_k(1), 3), ((64, 128), ev_v(1), None)]),
            (512, 64, [((0, 64), ev_v(0), None)]),
        ]
        # v pieces: v_h0 -> vvT[64:128, 0], v_h1 -> vvT[64:128, 1],
        # v_h2 -> vvT[0:64, 0] (from the m4 group, psum rows 0:64)

        proj_psums = {}

        def proj_unit(mi, n, kpair):
            """Two K-step matmuls of group (mi, n); evictions after the last."""
            c0, msz, evicts = mgroups[mi]
            key = (mi, n)
            if key not in proj_psums:
                proj_psums[key] = ps.tile([128, QT], F32, tag="ps", name="projp")
            p = proj_psums[key]
            for k in (2 * kpair, 2 * kpair + 1):
                nc.tensor.matmul(
                    p[:msz, :],
                    lhsT=wpk_sb[:, k, c0 : c0 + msz],
                    rhs=xT_sb[:, k, n * QT : (n + 1) * QT],
                    start=(k == 0),
                    stop=(k == NKD - 1),
                )
            if kpair == 2:
                del proj_psums[key]
                for (r0, r1), dst, bcol in evicts:
                    if mi == 4:
                        dst_ap = dst(n, 0, 64)  # v_h2 rows live at psum 0:64
                    else:
                        dst_ap = dst(n, r0, r1)
                    if bcol is None:
                        nc.vector.tensor_copy(out=dst_ap, in_=p[r0:r1, :])
                    else:
                        nc.vector.tensor_scalar_add(
                            out=dst_ap,
                            in0=p[r0:r1, :],
                            scalar1=bqk_sb[r0:r1, bcol : bcol + 1],
                        )

        def transpose_unit(t, piece):
            """piece 0/1/2 = head 0/1/2; v_h0/v_h1 at vvT[64:128,0/1], v_h2 at vvT[0:64,0]."""
            if piece == 2:
                src = vvT[0:64, 0, t * 128 : (t + 1) * 128]
                idn = ident[0:64, 0:64]
            else:
                src = vvT[64:128, piece, t * 128 : (t + 1) * 128]
                idn = ident[64:128, 64:128]
            pt = ps.tile([128, QT], F32, tag="ps")
            nc.tensor.transpose(pt[:, 0:64], src, idn)
            nc.vector.tensor_copy(v_aug[:, piece, t, 0:64], pt[:, 0:64])

        out_pair = out_p.rearrange("(tp a p) d -> tp p a d", a=2, p=128)
        o_pairs = {}

        def o_proj_unit(t, n2):
            key = t // 2
            if key not in o_pairs:
                o_pairs[key] = expp.tile([128, 2, D], F32, tag="osb", name="osb", bufs=2)
            ob = o_pairs[key]
            po = ps.tile([128, QT], F32, tag="ps")
            nc.tensor.matmul(
                po[:, 0:384],
                lhsT=zT01[:, t * 128 : (t + 1) * 128],
                rhs=wo_a[:, n2 * 384 : (n2 + 1) * 384],
                start=True,
                stop=False,
            )
            nc.tensor.matmul(
                po[:, 0:384],
                lhsT=zT2[:, t * 128 : (t + 1) * 128],
                rhs=wo_b[:, n2 * 384 : (n2 + 1) * 384],
                start=False,
                stop=True,
            )
            if t >= 12 and (t + n2) % 2 == 0:
                nc.scalar.activation(
                    out=ob[:, t % 2, n2 * 384 : (n2 + 1) * 384],
                    in_=po[:, 0:384],
                    func=AF.Copy,
                )
            else:
                nc.vector.tensor_copy(
                    out=ob[:, t % 2, n2 * 384 : (n2 + 1) * 384], in_=po[:, 0:384]
                )
            if t % 2 == 1 and n2 == 1:
                del o_pairs[key]
                nc.sync.dma_start(out=out_pair[key], in_=ob[:, :, :])

        # background work queue of (key, fn), drained between attention
        # iterations. Queue order is topological (a group's transposes come
        # after its evictions), so force-draining "through the last needed
        # unit" preserves all producer->consumer program ordering.
        work = deque()

        def q_proj(n, mis=range(5)):
            for mi in mis:
                for kpair in range(3):
                    work.append(
                        (("proj", n, mi), lambda mi=mi, n=n, kp=kpair: proj_unit(mi, n, kp))
                    )

        def q_tr(ts, pieces=range(HPC)):
            for t in ts:
                for piece in pieces:
                    work.append(
                        (("tr", t, piece), lambda t=t, p=piece: transpose_unit(t, p))
                    )

        def drain(k=1):
            for _ in range(k):
                if work:
                    work.popleft()[1]()

        def drain_all():
            while work:
                work.popleft()[1]()

        PROJ_GROUPS_FOR_HEAD = {0: (0, 1, 2), 1: (0, 2, 3), 2: (1, 3, 4)}

        def force_drain_for(h, qt):
            """Emit queued units up to the last one attention(h, qt) depends on."""
            needed = set()
            for n in range(qt + 1):
                for mi in PROJ_GROUPS_FOR_HEAD[h]:
                    needed.add(("proj", n, mi))
            for t in range(4 * qt + 4):
                needed.add(("tr", t, h))
            last = -1
            for i, (key, _) in enumerate(work):
                if key in needed:
                    last = i
            for _ in range(last + 1):
                work.popleft()[1]()

        def qh(h):
            m, off = divmod(h * 64, 128)
            return qT_sb[off : off + 64, m, :]

        def kh(h):
            m, off = divmod(h * 64, 128)
            return kT_sb[off : off + 64, m, :]

        zdst = [zT01[0:64, :], zT01[64:128, :], zT2[0:64, :]]

        # PV matmuls are pipelined ~4 iterations behind their exp across
        # block boundaries, so the in-order PE FIFO never waits on the
        # exp/mask chain, not even at the end of a block.
        pvq = deque()  # (block_serial, pv_closure)
        blk_serial = [0]

        def pv_drain(depth):
            while len(pvq) > depth:
                pvq.popleft()[1]()

        def pv_flush(upto_serial):
            while pvq and pvq[0][0] <= upto_serial:
                pvq.popleft()[1]()

        def attention(h, qt, per_kt):
            """scores^T -> exp -> causal mask -> PV into zp; diagonal blocks
            narrowed to q columns >= 128*r."""
            zp = psz.tile([DH + 1, QT], F32)
            nkt = 4 * qt + 4
            blk = blk_serial[0]
            blk_serial[0] += 1

            def pv(kt, es, lo):
                nc.tensor.matmul(
                    zp[:, lo:QT],
                    lhsT=v_aug[:, h, kt, :],
                    rhs=es[:, lo:QT],
                    start=(kt == 0),
                    stop=(kt == nkt - 1),
                )

            for kt in range(nkt):
                rr = kt - 4 * qt
                lo = 128 * rr if rr > 0 else 0
                sp = ps.tile([128, QT], F32, tag="ps")
                nc.tensor.matmul(
                    sp[:, lo:QT],
                    lhsT=kh(h)[:, kt * 128 : (kt + 1) * 128],
                    rhs=qh(h)[:, qt * QT + lo : (qt + 1) * QT],
                    start=True,
                    stop=True,
                )
                es = expp.tile([128, QT], F32R, tag="expp")
                nc.scalar.activation(out=es[:, lo:QT], in_=sp[:, lo:QT], func=AF.Exp)
                if rr >= 0:  # diagonal block: zero where key > query
                    nc.gpsimd.affine_select(
                        out=es[:, lo:QT],
                        in_=es[:, lo:QT],
                        compare_op=mybir.AluOpType.is_ge,
                        fill=0.0,
                        base=0,
                        channel_multiplier=-1,
                        pattern=[[1, QT - lo]],
                    )
                pvq.append((blk, lambda kt=kt, es=es, lo=lo: pv(kt, es, lo)))
                if per_kt == 2:
                    drain(2)
                elif per_kt == 9:
                    drain(1)
                elif kt % 2 == 0:
                    drain(1)
                pv_drain(4)
            return zp, blk

        def normalize(zp, h, qt):
            rec = small.tile([1, QT], F32R, tag="rec")
            with nc.allow_low_precision(reason="f32r is fp32-precision"):
                nc.vector.reciprocal(rec[:], zp[DH : DH + 1, :])
            bc = ps.tile([128, QT], F32, tag="ps")
            nc.tensor.matmul(
                bc[0:64, :], lhsT=ones64[:], rhs=rec[:], start=True, stop=True
            )
            bc_sb = small.tile([64, QT], F32, tag="bcsb")
            if qt == NQT - 1:
                nc.vector.tensor_copy(out=bc_sb[:], in_=bc[0:64, :])
            else:
                nc.scalar.activation(out=bc_sb[:], in_=bc[0:64, :], func=AF.Copy)
            nc.vector.tensor_mul(
                zdst[h][:, qt * QT : (qt + 1) * QT], zp[0:DH, :], bc_sb[:]
            )

        # ---- schedule ----
        # prologue: only what attention(h0, qt0) needs; the rest queues up.
        for mi in (0, 2, 1):
            for kpair in range(3):
                proj_unit(mi, 0, kpair)
        for t in range(4):
            transpose_unit(t, 0)
        q_proj(0, mis=(3,))
        q_tr(range(4), pieces=(1,))
        q_proj(0, mis=(4,))
        q_tr(range(4), pieces=(2,))
        for n in range(1, NQT):
            q_proj(n, mis=(0, 2, 1))
            q_tr(range(4 * n, 4 * n + 4), pieces=(0,))
            q_proj(n, mis=(3,))
            q_tr(range(4 * n, 4 * n + 4), pieces=(1,))
            q_proj(n, mis=(4,))
            q_tr(range(4 * n, 4 * n + 4), pieces=(2,))

        pending = None
        for qt in range(NQT):
            per_kt = 3 if qt == 0 else 1
            for h in range(HPC):
                force_drain_for(h, qt)
                zp, blk = attention(h, qt, per_kt)
                if pending is not None:
                    pv_flush(pending[3])  # pending block's PV accumulation done
                    normalize(*pending[:3])
                    ph, pqt = pending[1], pending[2]
                    if ph == HPC - 1:  # whole q-tile normalized -> O-proj ready
                        for t in range(4 * pqt, 4 * pqt + 4):
                            for n2 in range(2):
                                work.append(
                                    (("o", pqt), lambda t=t, n2=n2: o_proj_unit(t, n2))
                                )
                pending = (zp, h, qt, blk)
        pv_flush(pending[3])
        normalize(*pending[:3])
        drain_all()
        for t in range(4 * (NQT - 1), 4 * NQT):
            for n2 in range(2):
                o_proj_unit(t, n2)
    nc.finalize()
    return nc


_NC_CACHE = {}


def make_in_maps(x, W_qkv, b_qkv, W_o):
    in_maps = []
    for c in range(8):
        b, g = divmod(c, 4)
        hs = [HPC * g + i for i in range(HPC)]
        qr = [np.arange(64 * h, 64 * h + 64) for h in hs]
        w_q = [W_qkv[i] * 0.125 for i in qr]
        w_k = [W_qkv[768 + i] for i in qr]
        w_v = [W_qkv[1536 + i] for i in qr]
        b_q = [b_qkv[i] * 0.125 for i in qr]
        b_k = [b_qkv[768 + i] for i in qr]
        # packed rows: m0=[q0 q1] m1=[q2 v0] m2=[k0 k1] m3=[k2 v1] m4=[v2]
        wpk = np.concatenate(
            [w_q[0], w_q[1], w_q[2], w_v[0], w_k[0], w_k[1], w_k[2], w_v[1], w_v[2]],
            axis=0,
        )
        bqk_col = np.zeros((128, 4), np.float32)
        bqk_col[:, 0] = np.concatenate([b_q[0], b_q[1]])
        bqk_col[0:64, 1] = b_q[2]
        bqk_col[:, 2] = np.concatenate([b_k[0], b_k[1]])
        bqk_col[0:64, 3] = b_k[2]
        in_maps.append(
            {
                "xT": np.ascontiguousarray(x[b].T),
                "wpk": np.ascontiguousarray(wpk.T),
                "woT": np.ascontiguousarray(W_o[:, GD * g : GD * (g + 1)].T),
                "bqk": bqk_col,
                "vones": np.ones((128, 64), np.float32),
            }
        )
    return in_maps


def make_in_maps_for_test(inputs):
    return make_in_maps(
        np.asarray(inputs["x"], np.float32),
        np.asarray(inputs["W_qkv"], np.float32),
        np.asarray(inputs["b_qkv"], np.float32),
        np.asarray(inputs["W_o"], np.float32),
    )


def kernel(x, W_qkv, b_qkv, W_o, b_o):
    x = np.asarray(x, np.float32)
    W_qkv = np.asarray(W_qkv, np.float32)
    b_qkv = np.asarray(b_qkv, np.float32)
    W_o = np.asarray(W_o, np.float32)
    b_o = np.asarray(b_o, np.float32)

    if "nc" not in _NC_CACHE:
        _NC_CACHE["nc"] = build_bass()
    nc = _NC_CACHE["nc"]

    in_maps = make_in_maps(x, W_qkv, b_qkv, W_o)

    res = run_bass_kernel_spmd(
        nc,
        in_maps,
        list(range(8)),
        trace=bool(int(os.environ.get("KERNEL_TRACE", "0"))),
    )
    _NC_CACHE["last_results"] = res

    out = np.zeros((B, S, D), np.float32)
    for c in range(8):
        out[c // 4] += res.results[c]["out_p"]
    out += b_qkv[1536:] @ W_o.T + b_o
    return out


# revision 56
# speedup vs baseline: 1.0242x; 1.0005x over previous
"""Causal multi-head attention block (B=2, S=2048, D=768, H=12) on 8 trn2 cores.

Sharding: core c -> batch b = c//4 (data parallel), head group g = c%4
(tensor parallel, 3 heads per group). Each core computes its group's QKV
projection, causal attention, and a partial O-projection over its 192
z-columns. Host sums the 4 partials per batch and adds the biases that
commute through the math (v-bias and b_o).

On-core layout (everything "transposed", d on partitions, seq on free):
  xT   [768, 2048]   q/kT  [64*, 2048]      scores^T [keys, q]
so the softmax denominator comes free from a ones-column appended to V in
the PV matmul, and no on-chip transposes of activations are needed except
V (built via PE transpose from V^T).

The QKV projection uses a host-repacked weight matrix so every 128-wide
M-group is fully used:
  m0=[q_h0 q_h1] m1=[q_h2 v_h0] m2=[k_h0 k_h1] m3=[k_h2 v_h1] m4=[v_h2]
(q rows pre-scaled by 1/8; v bias folded into the host-side epilogue).

Matmul operands are float32r (full-rate fp32 on the PE). Scheduling
interleaves projection/transpose/O-proj work into the attention loop so
the scalar engine (exp) is never starved by a long PE FIFO stretch.
"""

import os
from collections import deque
from contextlib import ExitStack

import numpy as np

import concourse.tile as tile
from concourse import bacc, mybir
from concourse.bass_utils import run_bass_kernel_spmd
from concourse.masks import make_identity

F32 = mybir.dt.float32
F32R = mybir.dt.float32r
AF = mybir.ActivationFunctionType

B, S, D = 2, 2048, 768
NH, DH = 12, 64
HPC = 3            # heads per core
GD = HPC * DH      # 192 z-cols per core
KT, QT = 128, 512  # key tile (partitions), q tile (psum free)
NKT, NQT = S // KT, S // QT   # 16, 4
NTOK = S // 128    # 16 token tiles
NKD = D // 128     # 6 contraction tiles for the projections
WPK = 2 * GD + GD  # 576 packed projection rows


def build_bass():
    nc = bacc.Bacc(None)
    xT = nc.dram_tensor("xT", [D, S], F32, kind="ExternalInput")
    wpk = nc.dram_tensor("wpk", [D, WPK], F32, kind="ExternalInput")
    woT = nc.dram_tensor("woT", [GD, D], F32, kind="ExternalInput")
    bqk = nc.dram_tensor("bqk", [128, 4], F32, kind="ExternalInput")
    vones = nc.dram_tensor("vones", [128, 64], F32, kind="ExternalInput")
    out_p = nc.dram_tensor("out_p", [S, D], F32, kind="ExternalOutput")

    with tile.TileContext(nc) as tc, ExitStack() as ctx:
        const = ctx.enter_context(tc.tile_pool(name="const", bufs=1))
        ps = ctx.enter_context(tc.tile_pool(name="ps", bufs=6, space="PSUM"))
        psz = ctx.enter_context(tc.tile_pool(name="psz", bufs=2, space="PSUM"))
        expp = ctx.enter_context(tc.tile_pool(name="expp", bufs=9))
        small = ctx.enter_context(tc.tile_pool(name="small", bufs=4))

        xT_sb = const.tile([128, NKD, S], F32R)
        wpk_sb = const.tile([128, NKD, WPK], F32R)
        wo_a = const.tile([128, D], F32R)
        wo_b = const.tile([64, D], F32R)
        bqk_sb = const.tile([128, 4], F32)
        qT_sb = const.tile([128, 2, S], F32R)
        kT_sb = const.tile([128, 2, S], F32R)
        vvT = const.tile([128, 2, S], F32)
        v_aug = const.tile([128, HPC, NKT, DH + 1], F32R)
        zT01 = const.tile([128, S], F32R)
        zT2 = const.tile([64, S], F32R)
        ident = const.tile([128, 128], F32)
        ones64 = const.tile([1, 64], F32R)

        ones_stage = const.tile([128, 64], F32)
        make_identity(nc, ident[:])

        # ---- loads: k-interleaved so the first projection k-pairs unblock
        # early; everything not needed for (h0, qt0) comes after.
        xT_t = xT.rearrange("(t p) s -> t p s", p=128)
        wpk_t = wpk.rearrange("(t p) m -> t p m", p=128)
        for t in range(NKD):
            nc.sync.dma_start(
                out=wpk_sb[:, t, 0:384], in_=wpk_t[t][:, 0:384].bitcast(F32R)
            )
            nc.sync.dma_start(
                out=xT_sb[:, t, 0:QT], in_=xT_t[t][:, 0:QT].bitcast(F32R)
            )
        nc.sync.dma_start(out=bqk_sb[:], in_=bqk[:, :])
        for t in range(NKD):
            nc.sync.dma_start(
                out=wpk_sb[:, t, 384:WPK], in_=wpk_t[t][:, 384:WPK].bitcast(F32R)
            )
        nc.sync.dma_start(out=ones_stage[:], in_=vones[:, :])
        nc.sync.dma_start(out=ones64[:], in_=vones[0:1, 0:64].bitcast(F32R))
        nc.vector.tensor_copy(
            out=v_aug[:, :, :, DH],
            in_=ones_stage[:, 0 : HPC * NKT]
            .rearrange("p (h t) -> p h t", h=HPC)
            .bitcast(F32R),
        )
        for t in range(NKD):
            nc.sync.dma_start(
                out=xT_sb[:, t, QT : 2 * QT], in_=xT_t[t][:, QT : 2 * QT].bitcast(F32R)
            )
        nc.sync.dma_start(out=wo_a[:], in_=woT[0:128, :].bitcast(F32R))
        nc.sync.dma_start(out=wo_b[:], in_=woT[128:GD, :].bitcast(F32R))
        for t in range(NKD):
            nc.sync.dma_start(
                out=xT_sb[:, t, 2 * QT : S], in_=xT_t[t][:, 2 * QT : S].bitcast(F32R)
            )

        # packed projection m-groups: (col0, rows, evict spec)
        # evict spec: list of (psum row range, dst ap fn, bias col or None)
        def ev_q(col):
            return lambda n, r0, r1: qT_sb[r0:r1, col, n * QT : (n + 1) * QT]

        def ev_k(col):
            return lambda n, r0, r1: kT_sb[r0:r1, col, n * QT : (n + 1) * QT]

        def ev_v(col):
            return lambda n, r0, r1: vvT[r0:r1, col, n * QT : (n + 1) * QT]

        mgroups = [
            (0, 128, [((0, 128), ev_q(0), 0)]),
            (128, 128, [((0, 64), ev_q(1), 1), ((64, 128), ev_v(0), None)]),
            (256, 128, [((0, 128), ev_k(0), 2)]),
            (384, 128, [((0, 64), ev_k(1), 3), ((64, 128), ev_v(1), None)]),
            (512, 64, [((0, 64), ev_v(0), None)]),
        ]
        # v pieces: v_h0 -> vvT[64:128, 0], v_h1 -> vvT[64:128, 1],
        # v_h2 -> vvT[0:64, 0] (from the m4 group, psum rows 0:64)

        proj_psums = {}

        def proj_unit(mi, n, kpair):
            """Two K-step matmuls of group (mi, n); evictions after the last."""
            c0, msz, evicts = mgroups[mi]
            key = (mi, n)
            if key not in proj_psums:
                proj_psums[key] = ps.tile([128, QT], F32, tag="ps", name="projp")
            p = proj_psums[key]
            for k in (2 * kpair, 2 * kpair + 1):
                nc.tensor.matmul(
                    p[:msz, :],
                    lhsT=wpk_sb[:, k, c0 : c0 + msz],
                    rhs=xT_sb[:, k, n * QT : (n + 1) * QT],
                    start=(k == 0),
                    stop=(k == NKD - 1),
                )
            if kpair == 2:
                del proj_psums[key]
                for (r0, r1), dst, bcol in evicts:
                    if mi == 4:
                        dst_ap = dst(n, 0, 64)  # v_h2 rows live at psum 0:64
                    else:
                        dst_ap = dst(n, r0, r1)
                    if bcol is None:
                        nc.vector.tensor_copy(out=dst_ap, in_=p[r0:r1, :])
                    else:
                        nc.vector.tensor_scalar_add(
                            out=dst_ap,
                            in0=p[r0:r1, :],
                            scalar1=bqk_sb[r0:r1, bcol : bcol + 1],
                        )

        def transpose_unit(t, piece):
            """piece 0/1/2 = head 0/1/2; v_h0/v_h1 at vvT[64:128,0/1], v_h2 at vvT[0:64,0]."""
            if piece == 2:
                src = vvT[0:64, 0, t * 128 : (t + 1) * 128]
                idn = ident[0:64, 0:64]
            else:
                src = vvT[64:128, piece, t * 128 : (t + 1) * 128]
                idn = ident[64:128, 64:128]
            pt = ps.tile([128, QT], F32, tag="ps")
            nc.tensor.transpose(pt[:, 0:64], src, idn)
            nc.vector.tensor_copy(v_aug[:, piece, t, 0:64], pt[:, 0:64])

        out_pair = out_p.rearrange("(tp a p) d -> tp p a d", a=2, p=128)
        o_pairs = {}

        def o_proj_unit(t, n2):
            key = t // 2
            if key not in o_pairs:
                o_pairs[key] = expp.tile([128, 2, D], F32, tag="osb", name="osb", bufs=2)
            ob = o_pairs[key]
            po = ps.tile([128, QT], F32, tag="ps")
            nc.tensor.matmul(
                po[:, 0:384],
                lhsT=zT01[:, t * 128 : (t + 1) * 128],
                rhs=wo_a[:, n2 * 384 : (n2 + 1) * 384],
                start=True,
                stop=False,
            )
            nc.tensor.matmul(
                po[:, 0:384],
                lhsT=zT2[:, t * 128 : (t + 1) * 128],
                rhs=wo_b[:, n2 * 384 : (n2 + 1) * 384],
                start=False,
                stop=True,
            )
            if t >= 12 and (t + n2) % 2 == 0:
                nc.scalar.activation(
                    out=ob[:, t % 2, n2 * 384 : (n2 + 1) * 384],
                    in_=po[:, 0:384],
                    func=AF.Copy,
                )
            else:
                nc.vector.tensor_copy(
                    out=ob[:, t % 2, n2 * 384 : (n2 + 1) * 384], in_=po[:, 0:384]
                )
            if t % 2 == 1 and n2 == 1:
                del o_pairs[key]
                nc.sync.dma_start(out=out_pair[key], in_=ob[:, :, :])

        # background work queue of (key, fn), drained between attention
        # iterations. Queue order is topological (a group's transposes come
        # after its evictions), so force-draining "through the last needed
        # unit" preserves all producer->consumer program ordering.
        work = deque()

        def q_proj(n, mis=range(5)):
            for mi in mis:
                for kpair in range(3):
                    work.append(
                        (("proj", n, mi), lambda mi=mi, n=n, kp=kpair: proj_unit(mi, n, kp))
                    )

        def q_tr(ts, pieces=range(HPC)):
            for t in ts:
                for piece in pieces:
                    work.append(
                        (("tr", t, piece), lambda t=t, p=piece: transpose_unit(t, p))
                    )

        def drain(k=1):
            for _ in range(k):
                if work:
                    work.popleft()[1]()

        def drain_all():
            while work:
                work.popleft()[1]()

        PROJ_GROUPS_FOR_HEAD = {0: (0, 1, 2), 1: (0, 2, 3), 2: (1, 3, 4)}

        def force_drain_for(h, qt):
            """Emit queued units up to the last one attention(h, qt) depends on."""
            needed = set()
            for n in range(qt + 1):
                for mi in PROJ_GROUPS_FOR_HEAD[h]:
                    needed.add(("proj", n, mi))
            for t in range(4 * qt + 4):
                needed.add(("tr", t, h))
            last = -1
            for i, (key, _) in enumerate(work):
                if key in needed:
                    last = i
            for _ in range(last + 1):
                work.popleft()[1]()

        def qh(h):
            m, off = divmod(h * 64, 128)
            return qT_sb[off : off + 64, m, :]

        def kh(h):
            m, off = divmod(h * 64, 128)
            return kT_sb[off : off + 64, m, :]

        zdst = [zT01[0:64, :], zT01[64:128, :], zT2[0:64, :]]

        # PV matmuls are pipelined ~4 iterations behind their exp across
        # block boundaries, so the in-order PE FIFO never waits on the
        # exp/mask chain, not even at the end of a block.
        pvq = deque()  # (block_serial, pv_closure)
        blk_serial = [0]

        def pv_drain(depth):
            while len(pvq) > depth:
                pvq.popleft()[1]()

        def pv_flush(upto_serial):
            while pvq and pvq[0][0] <= upto_serial:
                pvq.popleft()[1]()

        def attention(h, qt, per_kt):
            """scores^T -> exp -> causal mask -> PV into zp; diagonal blocks
            narrowed to q columns >= 128*r."""
            zp = psz.tile([DH + 1, QT], F32)
            nkt = 4 * qt + 4
            blk = blk_serial[0]
            blk_serial[0] += 1

            def pv(kt, es, lo):
                nc.tensor.matmul(
                    zp[:, lo:QT],
                    lhsT=v_aug[:, h, kt, :],
                    rhs=es[:, lo:QT],
                    start=(kt == 0),
                    stop=(kt == nkt - 1),
                )

            for kt in range(nkt):
                rr = kt - 4 * qt
                lo = 128 * rr if rr > 0 else 0
                sp = ps.tile([128, QT], F32, tag="ps")
                nc.tensor.matmul(
                    sp[:, lo:QT],
                    lhsT=kh(h)[:, kt * 128 : (kt + 1) * 128],
                    rhs=qh(h)[:, qt * QT + lo : (qt + 1) * QT],
                    start=True,
                    stop=True,
                )
                es = expp.tile([128, QT], F32R, tag="expp")
                nc.scalar.activation(out=es[:, lo:QT], in_=sp[:, lo:QT], func=AF.Exp)
                if rr >= 0:  # diagonal block: zero where key > query
                    nc.gpsimd.affine_select(
                        out=es[:, lo:QT],
                        in_=es[:, lo:QT],
                        compare_op=mybir.AluOpType.is_ge,
                        fill=0.0,
                        base=0,
                        channel_multiplier=-1,
                        pattern=[[1, QT - lo]],
                    )
                pvq.append((blk, lambda kt=kt, es=es, lo=lo: pv(kt, es, lo)))
                if per_kt == 2:
                    drain(2)
                elif per_kt == 9:
                    drain(1)
                elif kt % 2 == 0:
                    drain(1)
                pv_drain(4)
            return zp, blk

        def normalize(zp, h, qt):
            rec = small.tile([1, QT], F32R, tag="rec")
            with nc.allow_low_precision(reason="f32r is fp32-precision"):
                nc.vector.reciprocal(rec[:], zp[DH : DH + 1, :])
            bc = ps.tile([128, QT], F32, tag="ps")
            nc.tensor.matmul(
                bc[0:64, :], lhsT=ones64[:], rhs=rec[:], start=True, stop=True
            )
            bc_sb = small.tile([64, QT], F32, tag="bcsb")
            if qt == NQT - 1:
                nc.vector.tensor_copy(out=bc_sb[:], in_=bc[0:64, :])
            else:
                nc.scalar.activation(out=bc_sb[:], in_=bc[0:64, :], func=AF.Copy)
            nc.vector.tensor_mul(
                zdst[h][:, qt * QT : (qt + 1) * QT], zp[0:DH, :], bc_sb[:]
            )

        # ---- schedule ----
        # prologue: only what attention(h0, qt0) needs; the rest queues up.
        for mi in (0, 2, 1):
            for kpair in range(3):
                proj_unit(mi, 0, kpair)
        for t in range(4):
            transpose_unit(t, 0)
        q_proj(0, mis=(3,))
        q_tr(range(4), pieces=(1,))
        q_proj(0, mis=(4,))
        q_tr(range(4), pieces=(2,))
        for n in range(1, NQT):
            q_proj(n, mis=(0, 2, 1))
            q_tr(range(4 * n, 4 * n + 4), pieces=(0,))
            q_proj(n, mis=(3,))
            q_tr(range(4 * n, 4 * n + 4), pieces=(1,))
            q_proj(n, mis=(4,))
            q_tr(range(4 * n, 4 * n + 4), pieces=(2,))

        pending = None
        for qt in range(NQT):
            per_kt = 3 if qt == 0 else 1
            for h in range(HPC):
                force_drain_for(h, qt)
                zp, blk = attention(h, qt, per_kt)
                if pending is not None:
                    pv_flush(pending[3])  # pending block's PV accumulation done
                    normalize(*pending[:3])
                    ph, pqt = pending[1], pending[2]
                    if ph == HPC - 1:  # whole q-tile normalized -> O-proj ready
                        for t in range(4 * pqt, 4 * pqt + 4):
                            for n2 in range(2):
                                work.append(
                                    (("o", pqt), lambda t=t, n2=n2: o_proj_unit(t, n2))
                                )
                pending = (zp, h, qt, blk)
        pv_flush(pending[3])
        normalize(*pending[:3])
        drain_all()
        for t in range(4 * (NQT - 1), 4 * NQT):
            for n2 in range(2):
                o_proj_unit(t, n2)
    nc.finalize()
    return nc


_NC_CACHE = {}


def make_in_maps(x, W_qkv, b_qkv, W_o):
    in_maps = []
    for c in range(8):
        b, g = divmod(c, 4)
        hs = [HPC * g + i for i in range(HPC)]
        qr = [np.arange(64 * h, 64 * h + 64) for h in hs]
        w_q = [W_qkv[i] * 0.125 for i in qr]
        w_k = [W_qkv[768 + i] for i in qr]
        w_v = [W_qkv[1536 + i] for i in qr]
        b_q = [b_qkv[i] * 0.125 for i in qr]
        b_k = [b_qkv[768 + i] for i in qr]
        # packed rows: m0=[q0 q1] m1=[q2 v0] m2=[k0 k1] m3=[k2 v1] m4=[v2]
        wpk = np.concatenate(
            [w_q[0], w_q[1], w_q[2], w_v[0], w_k[0], w_k[1], w_k[2], w_v[1], w_v[2]],
            axis=0,
        )
        bqk_col = np.zeros((128, 4), np.float32)
        bqk_col[:, 0] = np.concatenate([b_q[0], b_q[1]])
        bqk_col[0:64, 1] = b_q[2]
        bqk_col[:, 2] = np.concatenate([b_k[0], b_k[1]])
        bqk_col[0:64, 3] = b_k[2]
        in_maps.append(
            {
                "xT": np.ascontiguousarray(x[b].T),
                "wpk": np.ascontiguousarray(wpk.T),
                "woT": np.ascontiguousarray(W_o[:, GD * g : GD * (g + 1)].T),
                "bqk": bqk_col,
                "vones": np.ones((128, 64), np.float32),
            }
        )
    return in_maps


def make_in_maps_for_test(inputs):
    return make_in_maps(
        np.asarray(inputs["x"], np.float32),
        np.asarray(inputs["W_qkv"], np.float32),
        np.asarray(inputs["b_qkv"], np.float32),
        np.asarray(inputs["W_o"], np.float32),
    )


def kernel(x, W_qkv, b_qkv, W_o, b_o):
    x = np.asarray(x, np.float32)
    W_qkv = np.asarray(W_qkv, np.float32)
    b_qkv = np.asarray(b_qkv, np.float32)
    W_o = np.asarray(W_o, np.float32)
    b_o = np.asarray(b_o, np.float32)

    if "nc" not in _NC_CACHE:
        _NC_CACHE["nc"] = build_bass()
    nc = _NC_CACHE["nc"]

    in_maps = make_in_maps(x, W_qkv, b_qkv, W_o)

    res = run_bass_kernel_spmd(
        nc,
        in_maps,
        list(range(8)),
        trace=bool(int(os.environ.get("KERNEL_TRACE", "0"))),
    )
    _NC_CACHE["last_results"] = res

    out = np.zeros((B, S, D), np.float32)
    for c in range(8):
        out[c // 4] += res.results[c]["out_p"]
    out += b_qkv[1536:] @ W_o.T + b_o
    return out
